# revision 1
# baseline (speedup 1.0000x reference)
"""MiniBatchDiscrimination kernel for 8 Trainium2 NeuronCores.

Reference computation (N=256 samples, A=2048 in_features, B=64 out_features,
C=32 kernel dim):
    M  = (f @ T).reshape(N, B, C)
    L1[i,j,b] = sum_c |M[j,b,c] - M[i,b,c]|
    o[j,b]    = sum_i exp(-L1[i,j,b])        (includes the i==j self term = 1)
    out = concat([f, o], axis=1)

Strategy (retrieval-knn pruning):
  ||v||_1 >= ||v||_2, so a pair at squared-L2 distance D2 >= T_SCREEN
  (=2500, i.e. L2 >= 50) has L1 >= ~33 even after worst-case bf16 noise,
  and its exp(-L1) < 3e-15 contribution is invisible at any realistic
  tolerance (the reference's own fp32 terms underflow to exactly 0 for
  this data, where L1 ~ 1600).  D2 is computable on the TensorEngine at
  full speed via the Gram matrix:
      D2[i,j,b] = n[i,b] + n[j,b] - 2*G[i,j,b],   G = M_b @ M_b^T
  For N(0,1) random inputs D2 concentrates around 131k +- 33k (observed
  off-diagonal minimum 16.5k), so the only pairs below T_SCREEN are exact
  duplicates (D2 == 0, for which exp(-L1) == 1 exactly).  The device
  kernel therefore computes, for every (j, b):
      o[j,b] = #{ i : D2[i,j,b] < T_SCREEN }
  which equals the reference fp32 result whenever no pair falls in the
  ambiguous band 0 < L1 < ~50.  The host verifies this condition
  (o != 1 anywhere => some near-pair exists) and falls back to an exact
  host-side recomputation of the affected feature columns -- so the result
  is correct for ALL inputs; the fast path is exact for inputs without
  near-duplicate rows (up to a < 1e-12 relative error from dropped
  tiny terms).

Sharding: tensor-parallel over the B*C (=2048) columns of T.  Core d gets
T[:, 256*d : 256*(d+1)] (8 of the 64 b-features), computes M^T for its
block via PE (K=2048 GEMM), then Gram + screen for its 8 b's entirely
locally, and outputs o[:, 8d:8d+8].  No collectives; host concatenates.

Per-core device pipeline (all engines via TileContext auto-sync;
TimelineSim makespan ~20.5 us/core, HW steady-state throughput ~10 us):
  0. Host pre-tiles inputs to partition-major (f as fp8e4m3, T as
     bf16); chunked loads spread
     across the SP-HWDGE, ACT-HWDGE and Pool-SWDGE queues; dummy matmuls
     keep the PE p-state/HAM warm through the load phase.
  1. GEMM:  MT = (f @ Tblk)^T  as  out[bc, i] = sum_a Tblk[a, bc] * fT[a, i]
     (lhsT = bf16 Tblk tiles, rhs = fp8 fT tiles, fp32 PSUM accumulate)
  2. sq = MT^2 (bf16, on GPSIMD); norms in both orientations on PE:
     rows n_b[j] via S128 (lands -n/2 directly at partitions 32*(b%4),
     the quadrant the rank-1 fold needs) and cols n_i via S (128,4),
     both t-halves packed into one PSUM tile per i-half -> single
     threshold copies  tsc = (n_i - T)/2, tscn = -tsc.
  3. Per (b, i-half):  PSUM = Gram (K=32 at tile_position 32*(b%4))
     + (-n_j/2) rank-1 fold (K=1 ones x norm row, same quadrant)
  4. indicator + count in ONE op per (b, i-half):  compare against the
     per-partition threshold with free-dim accum_out (D2 symmetric =>
     row count == col count); even b -> DVE is_gt, odd b -> ACT Sign
     (+ one strided fixup), engines alternating per gram group
  5. single DMA of o packed (128, 16) fp32 out; host unpacks/concats.
"""

import os

import ml_dtypes
import numpy as np

N = 256  # batch
A = 2048  # in_features
B = 64  # out_features
C = 32  # kernel dim
NCORES = 8
BLOCAL = B // NCORES  # 8 b-features per core
BCL = BLOCAL * C  # 256 M^T rows per core
KT = A // 128  # 16 k-tiles
# Squared-L2 screen threshold.  Pairs with computed D2 >= T are dropped.
# Quantization error (fp8 f + bf16 T/M) is distance-proportional: identical
# rows compute D2 ~ 1e2 << T, and computed D2 >= T still implies true
# L1 >= ~34 => dropped contribution < 2e-15.  Measured minimum computed
# off-diagonal D2 is 1.67e4, 6.7x above T, so quantization noise cannot
# produce a false survivor (which would only cost a host fallback anyway).
T_SCREEN = 2500.0

_BF16 = ml_dtypes.bfloat16

_compiled = None
last_run_info = None  # BassKernelResults of the most recent device run


def _emit_body(nc, mybir, inp, work, scr, pbig, pn, consts, fT_d, Tb_d, o_d):
    f32 = mybir.dt.float32
    bf16 = mybir.dt.bfloat16
    S_sb, S128_sb, ones_sb = consts

    # ---- load inputs, chunked so the GEMM starts after the first chunk ----
    # (row a*128+p of DRAM -> tile [p, a, :]); small first chunks so k-tile 0
    # arrives early, spread across three DMA queues
    SIZES = [2, 4, 5, 5]  # k-tiles per chunk
    offs = [sum(SIZES[:i]) for i in range(len(SIZES))]
    kt2chunk = [
        (c, k - offs[c])
        for k in range(KT)
        for c in range(len(SIZES))
        if offs[c] <= k < offs[c] + SIZES[c]
    ]
    fT_ch, Tb_ch = [], []
    fT_q = [nc.sync] * 4
    Tb_q = [nc.gpsimd, nc.scalar, nc.scalar, nc.scalar]
    for c, sz in enumerate(SIZES):
        ftt = inp.tile([128, sz, N], mybir.dt.float8e4, tag=f"fT{c}")
        fT_q[c].dma_start(
            ftt[:], fT_d[:, N * offs[c] : N * (offs[c] + sz)]
        )
        fT_ch.append(ftt)
        tbt = inp.tile([128, sz, BCL], bf16, tag=f"Tb{c}")
        Tb_q[c].dma_start(
            tbt[:], Tb_d[:, BCL * offs[c] : BCL * (offs[c] + sz)]
        )
        Tb_ch.append(tbt)

    # PE pstate warmup: keep the array busy during the load phase so the
    # first real matmuls run at full clock (same trick warms the HW HAM)
    wp = pn.tile([128, 128], f32, tag="nr", name="wp", bufs=1)
    for w in range(16):
        nc.tensor.matmul(
            wp[:],
            ones_sb[0:1, :],
            ones_sb[0:1, :],
            start=(w == 0),
            stop=(w == 15),
        )

    # o packed (128, 16): column 8*mt + b  (single out-DMA)
    o_sb = work.tile([128, 2 * BLOCAL], f32, tag="o")
    tsc_sb = [work.tile([128, 8], f32, tag=f"tsc{mt}", name=f"tsc{mt}") for mt in range(2)]
    tscn_sb = [work.tile([128, 8], f32, tag=f"tscn{mt}", name=f"tscn{mt}") for mt in range(2)]

    msb_l, ssb_l, nrt_l = [], [], []
    ncp_l = [pn.tile([128, 8], f32, tag=f"ncol{mt}", name=f"ncp{mt}", bufs=1) for mt in range(2)]

    def emit_gemm(t):
        mtp = pbig.tile([128, N], f32, tag="big", name=f"mtp{t}")
        for kt in range(KT):
            c, j = kt2chunk[kt]
            nc.tensor.matmul(
                mtp[:],
                Tb_ch[c][:, j, 128 * t : 128 * (t + 1)],
                fT_ch[c][:, j, :],
                start=(kt == 0),
                stop=(kt == KT - 1),
            )
        msb = scr.tile([128, N], bf16, tag=f"mt{t}", name=f"msb{t}")
        nc.vector.tensor_copy(msb[:], mtp[:])
        # squares straight from PSUM on ACT (shorter norm-chain; the n-vs-Gram
        # inconsistency is O(300), far inside the screening margin)
        ssb = scr.tile([128, N], bf16, tag=f"sq{t}", name=f"ssb{t}")
        nc.scalar.square(ssb[:], mtp[:])
        msb_l.append(msb)
        ssb_l.append(ssb)

    def emit_norms(t):
        ssb = ssb_l[t]
        # norm rows: S128 places -n_b/2 at partition 32*(b%4) directly
        # (rank-1 fold operands must sit in the matmul's row quadrant)
        npp = pn.tile([128, N], f32, tag="nr", bufs=1, name=f"npp{t}")
        nc.tensor.matmul(npp[:], S128_sb[:], ssb[:], start=True, stop=True)
        nrt = work.tile([128, N], bf16, tag=f"nrow{t}", name=f"nrt{t}")
        nc.scalar.mul(nrt[:], npp[:], -0.5)
        nrt_l.append(nrt)
        # per-partition norm cols: both t's land in one psum tile per mt
        for mt in range(2):
            nc.tensor.matmul(
                ncp_l[mt][:, 4 * t : 4 * t + 4],
                ssb[:, 128 * mt : 128 * (mt + 1)],
                S_sb[:],
                start=True,
                stop=True,
            )

    def emit_thresholds():
        # tsc = (n_i - T)/2 ; tscn = -tsc  (one copy pair per mt)
        for mt in range(2):
            nc.scalar.activation(
                tsc_sb[mt][:],
                ncp_l[mt][:],
                mybir.ActivationFunctionType.Copy,
                bias=-T_SCREEN / 2.0,
                scale=0.5,
            )
            nc.scalar.activation(
                tscn_sb[mt][:],
                ncp_l[mt][:],
                mybir.ActivationFunctionType.Copy,
                bias=T_SCREEN / 2.0,
                scale=-0.5,
            )

    def emit_gram_group(g, t):
        msb, nrt = msb_l[t], nrt_l[t]
        if True:
            b = 4 * t + g
            for mt in range(2):
                gp = pbig.tile([128, N], f32, tag="big")
                nc.tensor.matmul(
                    gp[:],
                    msb[32 * g : 32 * g + 32, 128 * mt : 128 * (mt + 1)],
                    msb[32 * g : 32 * g + 32, :],
                    start=True,
                    stop=False,
                    tile_position=(32 * g, 0),
                )
                nc.tensor.matmul(
                    gp[:],
                    ones_sb[32 * g : 32 * g + 1, :],
                    nrt[32 * g : 32 * g + 1, :],
                    start=False,
                    stop=True,
                    tile_position=(32 * g, 0),
                )
                if b % 2 == 0:
                    # DVE: ind = (G' > tsc_i), count = sum_j ind
                    ind = scr.tile([128, N], bf16, tag="ind")
                    nc.vector.tensor_scalar(
                        ind[:],
                        gp[:],
                        tsc_sb[mt][:, b : b + 1],
                        None,
                        mybir.AluOpType.is_gt,
                        mybir.AluOpType.add,
                        accum_out=o_sb[:, 8 * mt + b : 8 * mt + b + 1],
                    )
                else:
                    # ACT: sign(G' - tsc_i) summed; fixed up below
                    ind = scr.tile([128, N], f32, tag="inda")
                    nc.scalar.activation(
                        ind[:],
                        gp[:],
                        mybir.ActivationFunctionType.Sign,
                        bias=tscn_sb[mt][:, b : b + 1],
                        scale=1.0,
                        accum_out=o_sb[:, 8 * mt + b : 8 * mt + b + 1],
                    )

    # pipelined order: t0 screen overlaps t1 GEMM on DVE/ACT
    emit_gemm(0)
    emit_norms(0)
    emit_gemm(1)
    emit_norms(1)
    emit_thresholds()
    for t in range(2):
        for g in range(4):
            emit_gram_group(g, t)

    # ACT columns (odd b -> odd cols) hold sum(sign); count = (x + N) / 2
    nc.vector.tensor_scalar(
        o_sb[:, 1 : 2 * BLOCAL : 2],
        o_sb[:, 1 : 2 * BLOCAL : 2],
        0.5,
        float(N) * 0.5,
        mybir.AluOpType.mult,
        mybir.AluOpType.add,
    )
    nc.sync.dma_start(o_d[:], o_sb[:])


def _build(chain=False, reps=1):
    import concourse.mybir as mybir
    import concourse.tile as tile
    from concourse import bacc

    f32 = mybir.dt.float32
    bf16 = mybir.dt.bfloat16

    nc = bacc.Bacc(None, target_bir_lowering=False, debug=False)
    # host pre-tiles to partition-major: row p holds [x[kt*128+p, :] for kt]
    # f ships as fp8e4m3: D2 error stays distance-proportional (identical
    # rows -> D2 ~ 128 << 2500; computed D2 >= 2500 still implies true
    # L1 >= ~34), measured min computed off-diag D2 = 16.7k vs T = 2.5k
    fT_d = nc.dram_tensor("fT", [128, KT * N], mybir.dt.float8e4, kind="ExternalInput")
    Tb_d = nc.dram_tensor("Tb", [128, KT * BCL], bf16, kind="ExternalInput")
    o_d = nc.dram_tensor("o", [128, 2 * BLOCAL], f32, kind="ExternalOutput")
    if chain:
        # benchmark-only: data-dependent passthrough for chaining execs
        ch_i = nc.dram_tensor("chain", [128, 16], f32, kind="ExternalInput")
        ch_o = nc.dram_tensor("chain_out", [128, 16], f32, kind="ExternalOutput")
    if reps != 1:
        # bench-only builds must not share the production build's HLO
        # signature (the NEFF cache keys on I/O signature alone)
        nc.dram_tensor("repstag", [1, 16 + reps], f32, kind="ExternalInput")

    with tile.TileContext(nc) as tc:
        with (
            tc.tile_pool(name="inp", bufs=2) as inp,
            tc.tile_pool(name="work", bufs=1) as work,
            tc.tile_pool(name="scr", bufs=3) as scr,
            tc.tile_pool(name="pbig", bufs=5, space="PSUM") as pbig,
            tc.tile_pool(name="pn", bufs=2, space="PSUM") as pn,
        ):
            if chain:
                cht = work.tile([128, 16], f32, tag="chain")
                nc.sync.dma_start(cht[:], ch_i[:])
                nc.sync.dma_start(ch_o[:], cht[:])
            # block-indicator consts: S[p, g] = 1 iff p//32 == g, and the
            # 128-wide variant with column 32g live so norm-matmul output
            # rows land at 32-aligned partitions
            S_sb = work.tile([128, 4], bf16, tag="S")
            nc.vector.memset(S_sb[:], 0.0)
            S128_sb = work.tile([128, 128], bf16, tag="S128")
            nc.vector.memset(S128_sb[:], 0.0)
            for g in range(4):
                nc.vector.memset(S_sb[32 * g : 32 * g + 32, g : g + 1], 1.0)
                nc.vector.memset(
                    S128_sb[32 * g : 32 * g + 32, 32 * g : 32 * g + 1], 1.0
                )
            # ones rows at every 32-aligned partition (stationary for the
            # rank-1 -n_j/2 fold; quadrant must match the norm-row quadrant)
            ones_sb = work.tile([128, 128], bf16, tag="ones")
            nc.vector.memset(ones_sb[:], 1.0)

            for _rep in range(reps):
                _emit_body(
                    nc, mybir, inp, work, scr, pbig, pn,
                    (S_sb, S128_sb, ones_sb), fT_d, Tb_d, o_d,
                )

    nc.compile()
    return nc


def _get_compiled():
    global _compiled
    if _compiled is None:
        _compiled = _build()
    return _compiled


def _host_exact_o_column(f64, T64, b):
    """Exact (float64) o[:, b] for one feature column; used only when the
    device screen detects a potential near-duplicate pair."""
    Mb = f64 @ T64[:, C * b : C * (b + 1)]  # (N, C)
    L1 = np.abs(Mb[None, :, :] - Mb[:, None, :]).sum(axis=2)  # (N, N)
    return np.exp(-L1).sum(axis=0)


def _tile_rows(x):
    """(A, W) row-major -> (128, KT*W) partition-major (row p = k-tiles concat)."""
    w = x.shape[1]
    return np.ascontiguousarray(
        x.reshape(KT, 128, w).transpose(1, 0, 2).reshape(128, KT * w)
    )


def make_in_maps(f, T):
    fT = _tile_rows(f.T.astype(ml_dtypes.float8_e4m3))
    return [
        {
            "fT": fT,
            "Tb": _tile_rows(
                T[:, BCL * d : BCL * (d + 1)].astype(_BF16)
            ),
        }
        for d in range(NCORES)
    ]


def kernel(f, T):
    from concourse.bass_utils import run_bass_kernel_spmd

    global last_run_info
    f = np.asarray(f)
    T = np.asarray(T)
    assert f.shape == (N, A) and T.shape == (A, B * C), (f.shape, T.shape)

    nc = _get_compiled()
    in_maps = make_in_maps(f, T)
    res = run_bass_kernel_spmd(
        nc,
        in_maps,
        core_ids=list(range(NCORES)),
        trace=bool(int(os.environ.get("KERNEL_TRACE", "0"))),
    )
    last_run_info = res

    o = np.empty((N, B), dtype=np.float32)
    for d in range(NCORES):
        od = res.results[d]["o"].reshape(128, 2, BLOCAL)
        o[:, BLOCAL * d : BLOCAL * (d + 1)] = od.transpose(1, 0, 2).reshape(
            N, BLOCAL
        )

    # Screen verification: counts other than 1.0 mean either true duplicates
    # (count k of an identical group => reference sum is also k: exact) or a
    # near-pair in the ambiguous band.  Distinguishing costs more than an
    # exact host recompute of the affected columns, so just recompute those.
    bad_cols = np.where(np.any(o != 1.0, axis=0))[0]
    if bad_cols.size:
        f64 = f.astype(np.float64)
        T64 = T.astype(np.float64)
        for b in bad_cols:
            o[:, b] = _host_exact_o_column(f64, T64, int(b)).astype(np.float32)

    return np.concatenate([f.astype(np.float32, copy=False), o], axis=1)



# revision 9
# speedup vs baseline: 1.1509x; 1.1509x over previous
"""MiniBatchDiscrimination kernel for 8 Trainium2 NeuronCores.

Reference computation (N=256 samples, A=2048 in_features, B=64 out_features,
C=32 kernel dim):
    M  = (f @ T).reshape(N, B, C)
    L1[i,j,b] = sum_c |M[j,b,c] - M[i,b,c]|
    o[j,b]    = sum_i exp(-L1[i,j,b])        (includes the i==j self term = 1)
    out = concat([f, o], axis=1)

Strategy (retrieval-knn pruning, see kernel_v1_backup.py for the full
derivation): ||v||_1 >= ||v||_2, so the squared-L2 screen
    D2[i,j,b] = n[i,b] + n[j,b] - 2*G[i,j,b]  (G = Gram of M_b)
with threshold T_SCREEN certifies that every dropped pair contributes
< 3e-15 to o.  For this input class the only survivors are the diagonal
(count 1 == exact fp32 reference).  The host verifies (any o != 1 =>
exact recompute of the affected columns), so the result is correct for
ALL inputs.

Sharding: tensor-parallel over the B*C columns of T: core d computes
o[:, 8d:8d+8] with no collectives.

v2 device pipeline per core (everything fp8 in, cost-model-guided):
  - f and T ship as fp8e4m3, partition-major k-tiles.  4 HWDGE loads.
  - GEMM M^T = (f @ Tblk)^T via DoubleRow fp8 matmuls (2 k-tiles per
    instruction, 0.5 cycles/row): 8 matmuls per 128-row output half.
  - per half t: msb = bf16 copy of M^T; ssb = M^2 (bf16); norm rows
    nrow = (n_b[j] - T)/2 at partition 32g via S128(0.5) matmul + bias;
    norm cols n_b[i]/2 via S(0.5) matmul -> small SBUF copy (ncps).
  - per (b, i-half): Gram (K=32, quadrant-packed).  Indicators:
      DVE/Pool: scalar_tensor_tensor (G - n_i/2) > nbc, where nbc is a
        rank-1 broadcast PSUM tile of (n_j - T)/2 (one per b, shared by
        both halves); accum_out -> count column.
      ACT: Sign(n_i/2 - Gf) on a fold-accumulated Gram (Gf = G -
        (n_j-T)/2 via a -1s x nrow rank-1), accum_out -> (256-x)/2
        fixed up on the host.
  - counts accumulate straight into o_sb [128, 16]; single DMA out.
"""

import os

import ml_dtypes
import numpy as np

N = 256  # batch
A = 2048  # in_features
B = 64  # out_features
C = 32  # kernel dim
NCORES = 8
BLOCAL = B // NCORES  # 8 b-features per core
BCL = BLOCAL * C  # 256 M^T rows per core
KT = A // 128  # 16 k-tiles
# Squared-L2 screen threshold: measured min off-diagonal computed D2 for
# fp8 f AND fp8 T is 1.64e4, 6.5x above T_SCREEN; identical rows compute
# D2 ~ 1e2 << T.  Computed D2 >= T still implies true L1 >= ~34.
T_SCREEN = 2500.0

_FP8 = ml_dtypes.float8_e4m3

# indicator engine per (t, g, mt) in emission order; 'D' = DVE is_gt,
# 'P' = Pool is_gt, 'A' = ACT Sign (host fixes counts up).  All read the
# fold-accumulated Gram (Gf = G - (n_j - T)/2) with per-partition n_i/2.
_PATTERN = [
    "D", "A", "D", "A", "D", "A", "D", "D",  # t=0: (g,mt) pairs in order
    "A", "D", "A", "D", "D", "A", "D", "A",  # t=1; last group ends on D+A
]
ENG_ASSIGN = {
    (t, g, mt): _PATTERN[8 * t + 2 * g + mt]
    for t in range(2)
    for g in range(4)
    for mt in range(2)
}

_compiled = None
last_run_info = None


def _col(t, g, mt):
    return 8 * mt + 4 * t + g


def _emit_body(nc, mybir, inp, work, scr, pbig, pn, consts, fT_d, Tb_d, o_d):
    f32 = mybir.dt.float32
    bf16 = mybir.dt.bfloat16
    fp8 = mybir.dt.float8e4
    S_sb, S128_sb, ones_sb, mones_sb = consts

    # ---- input loads: 4 chunks (fT x2, Tb x2) on two HWDGE queues ----
    SIZES = [8, 8]
    offs = [0, 8]
    fT_ch, Tb_ch = [], []
    for c, sz in enumerate(SIZES):
        ftt = inp.tile([128, sz, N], fp8, tag=f"fT{c}", name=f"ftt{c}")
        nc.sync.dma_start(ftt[:], fT_d[:, N * offs[c] : N * (offs[c] + sz)])
        fT_ch.append(ftt)
        tbt = inp.tile([128, sz, BCL], fp8, tag=f"Tb{c}", name=f"tbt{c}")
        nc.scalar.dma_start(tbt[:], Tb_d[:, BCL * offs[c] : BCL * (offs[c] + sz)])
        Tb_ch.append(tbt)

    # PE pstate warmup: get pe_busy_start ticking early so real matmuls hit
    # full clock once ramp > 3us
    wp = pn.tile([128, 128], f32, tag="pn", name="wp", bufs=1)
    for w in range(8):
        nc.tensor.matmul(
            wp[:],
            ones_sb[0:1, 0:128],
            ones_sb[0:1, 0:128],
            start=(w == 0),
            stop=(w == 7),
        )

    o_sb = work.tile([128, 2 * BLOCAL], f32, tag="o")

    msb_l, nrow_l = [], []
    # n_i/2 per (mt, t, g) at ncps[:, mt, t, g]
    ncps = work.tile([128, 2, 2, 4], f32, tag="ncps")

    def emit_gemm(t):
        mtp = pbig.tile([128, N], f32, tag="mtp", bufs=2, name=f"mtp{t}")
        k = 0
        for c, sz in enumerate(SIZES):
            for j in range(sz // 2):
                nc.tensor.matmul(
                    mtp[:],
                    Tb_ch[c][:, 2 * j : 2 * j + 2, 128 * t : 128 * (t + 1)],
                    fT_ch[c][:, 2 * j : 2 * j + 2, :],
                    start=(k == 0),
                    stop=(k == KT // 2 - 1),
                    perf_mode=mybir.MatmulPerfMode.DoubleRow,
                )
                k += 1
        return mtp

    def emit_norms(t, mtp):
        # bf16 M^T for the Gram
        msb = scr.tile([128, N], bf16, tag=f"mt{t}", name=f"msb{t}")
        nc.vector.tensor_copy(msb[:], mtp[:])
        msb_l.append(msb)
        # squares (bf16) for the norm matmuls
        ssb = scr.tile([128, N], bf16, tag=f"sq{t}", name=f"ssb{t}")
        nc.scalar.square(ssb[:], mtp[:])
        # norm rows: S128 holds 0.5 at (c-rows of g, col 32g) ->
        # npp[32g, j] = n_{b=4t+g}[j]/2
        npp = pn.tile([128, N], f32, tag="pn", bufs=1, name=f"npp{t}")
        nc.tensor.matmul(npp[:], S128_sb[:], ssb[:], start=True, stop=True)
        # nrow = npp - T/2  (= (n_j - T)/2 at partition 32g); GPSIMD cannot
        # read PSUM, so this lands on DVE/ACT
        nrow = work.tile([128, N], bf16, tag=f"nrow{t}", name=f"nrow{t}")
        if t == 0:
            nc.scalar.activation(
                nrow[:],
                npp[:],
                mybir.ActivationFunctionType.Copy,
                bias=-T_SCREEN / 2.0,
                scale=1.0,
            )
        else:
            nc.vector.tensor_scalar(
                nrow[:], npp[:], -T_SCREEN / 2.0, None, mybir.AluOpType.add
            )
        nrow_l.append(nrow)
        # norm cols: ncp[i, mt, g] = n_{4t+g}[i]/2 for i in half mt
        ncp = pn.tile([128, 2, 4], f32, tag="ncp", bufs=1, name=f"ncp{t}")
        for mt in range(2):
            nc.tensor.matmul(
                ncp[:, mt, :],
                ssb[:, 128 * mt : 128 * (mt + 1)],
                S_sb[:],
                start=True,
                stop=True,
            )
        # -> SBUF (STT scalar / ACT bias want SBUF)
        nc.vector.tensor_copy(ncps[:, :, t, :], ncp[:])

    def emit_screen(t):
        msb, nrow = msb_l[t], nrow_l[t]
        for g in range(4):
            for mt in range(2):
                eng = ENG_ASSIGN[(t, g, mt)]
                col = _col(t, g, mt)
                # Gf = G - (n_j - T)/2: Gram + (-1s x nrow) rank-1 fold
                gp = pbig.tile([128, N], f32, tag="big", bufs=3)
                nc.tensor.matmul(
                    gp[:],
                    msb[32 * g : 32 * g + 32, 128 * mt : 128 * (mt + 1)],
                    msb[32 * g : 32 * g + 32, :],
                    start=True,
                    stop=False,
                    tile_position=(32 * g, 0),
                )
                nc.tensor.matmul(
                    gp[:],
                    mones_sb[32 * g : 32 * g + 1, 0:128],
                    nrow[32 * g : 32 * g + 1, :],
                    start=False,
                    stop=True,
                    tile_position=(32 * g, 0),
                )
                # D2 < T  <=>  Gf > n_i/2
                if eng == "A":
                    # sign(n_i/2 - Gf) = sign((D2 - T)/2); host: (256 - x)/2
                    ind = scr.tile([128, N], f32, tag="inda", name="ind")
                    nc.scalar.activation(
                        ind[:],
                        gp[:],
                        mybir.ActivationFunctionType.Sign,
                        bias=ncps[:, mt, t, g : g + 1],
                        scale=-1.0,
                        accum_out=o_sb[:, col : col + 1],
                    )
                else:
                    ind = scr.tile([128, N], bf16, tag="ind", name="ind")
                    e = nc.vector if eng == "D" else nc.gpsimd
                    e.tensor_scalar(
                        ind[:],
                        gp[:],
                        ncps[:, mt, t, g : g + 1],
                        None,
                        mybir.AluOpType.is_gt,
                        mybir.AluOpType.add,
                        accum_out=o_sb[:, col : col + 1],
                    )

    mtp0 = emit_gemm(0)
    mtp1 = emit_gemm(1)
    emit_norms(0, mtp0)
    emit_norms(1, mtp1)
    emit_screen(0)
    emit_screen(1)

    nc.sync.dma_start(o_d[:], o_sb[:])


def _build():
    import concourse.mybir as mybir
    import concourse.tile as tile
    from concourse import bacc

    f32 = mybir.dt.float32
    bf16 = mybir.dt.bfloat16
    fp8 = mybir.dt.float8e4

    nc = bacc.Bacc(None, target_bir_lowering=False, debug=False)
    fT_d = nc.dram_tensor("fT", [128, KT * N], fp8, kind="ExternalInput")
    Tb_d = nc.dram_tensor("Tb", [128, KT * BCL], fp8, kind="ExternalInput")
    o_d = nc.dram_tensor("o", [128, 2 * BLOCAL], f32, kind="ExternalOutput")

    with tile.TileContext(nc) as tc:
        with (
            tc.tile_pool(name="inp", bufs=1) as inp,
            tc.tile_pool(name="work", bufs=1) as work,
            tc.tile_pool(name="scr", bufs=2) as scr,
            tc.tile_pool(name="pbig", bufs=1, space="PSUM") as pbig,
            tc.tile_pool(name="pn", bufs=1, space="PSUM") as pn,
        ):
            # S_sb[p, g] = 0.5 iff p//32 == g; S128: 0.5 at (rows of g,
            # col 32g) so norm rows land at partition 32g
            S_sb = work.tile([128, 4], bf16, tag="S")
            nc.vector.memset(S_sb[:], 0.0)
            S128_sb = work.tile([128, 128], bf16, tag="S128")
            nc.vector.memset(S128_sb[:], 0.0)
            for g in range(4):
                nc.vector.memset(S_sb[32 * g : 32 * g + 32, g : g + 1], 0.5)
                nc.vector.memset(
                    S128_sb[32 * g : 32 * g + 32, 32 * g : 32 * g + 1], 0.5
                )
            ones_sb = work.tile([128, 128], bf16, tag="ones")
            nc.vector.memset(ones_sb[:], 1.0)
            mones_sb = work.tile([128, 128], bf16, tag="mones")
            nc.gpsimd.memset(mones_sb[:], -1.0)

            _emit_body(
                nc, mybir, inp, work, scr, pbig, pn,
                (S_sb, S128_sb, ones_sb, mones_sb), fT_d, Tb_d, o_d,
            )

    nc.compile()
    return nc


def _get_compiled():
    global _compiled
    if _compiled is None:
        _compiled = _build()
    return _compiled


def _host_exact_o_column(f64, T64, b):
    """Exact (float64) o[:, b] for one feature column; used only when the
    device screen detects a potential near-duplicate pair."""
    Mb = f64 @ T64[:, C * b : C * (b + 1)]  # (N, C)
    L1 = np.abs(Mb[None, :, :] - Mb[:, None, :]).sum(axis=2)  # (N, N)
    return np.exp(-L1).sum(axis=0)


def _tile_rows(x):
    """(A, W) row-major -> (128, KT*W) partition-major (row p = k-tiles concat)."""
    w = x.shape[1]
    return np.ascontiguousarray(
        x.reshape(KT, 128, w).transpose(1, 0, 2).reshape(128, KT * w)
    )


def make_in_maps(f, T):
    fT = _tile_rows(f.T.astype(_FP8))
    return [
        {
            "fT": fT,
            "Tb": _tile_rows(T[:, BCL * d : BCL * (d + 1)].astype(_FP8)),
        }
        for d in range(NCORES)
    ]


def kernel(f, T):
    from concourse.bass_utils import run_bass_kernel_spmd

    global last_run_info
    f = np.asarray(f)
    T = np.asarray(T)
    assert f.shape == (N, A) and T.shape == (A, B * C), (f.shape, T.shape)

    nc = _get_compiled()
    in_maps = make_in_maps(f, T)
    res = run_bass_kernel_spmd(
        nc,
        in_maps,
        core_ids=list(range(NCORES)),
        trace=bool(int(os.environ.get("KERNEL_TRACE", "0"))),
    )
    last_run_info = res

    o = np.empty((N, B), dtype=np.float32)
    for d in range(NCORES):
        od = np.array(res.results[d]["o"])  # [128, 16]
        # ACT Sign columns hold sum(sign((D2-T)/2)) = 256 - 2*count
        for (t, g, mt), eng in ENG_ASSIGN.items():
            if eng == "A":
                c = _col(t, g, mt)
                od[:, c] = (256.0 - od[:, c]) * 0.5
        od = od.reshape(128, 2, BLOCAL)
        o[:, BLOCAL * d : BLOCAL * (d + 1)] = od.transpose(1, 0, 2).reshape(
            N, BLOCAL
        )

    # Screen verification: any count != 1 => exact host recompute of that col.
    bad_cols = np.where(np.any(o != 1.0, axis=0))[0]
    if bad_cols.size:
        f64 = f.astype(np.float64)
        T64 = T.astype(np.float64)
        for b in bad_cols:
            o[:, b] = _host_exact_o_column(f64, T64, int(b)).astype(np.float32)

    return np.concatenate([f.astype(np.float32, copy=False), o], axis=1)


# revision 24
# speedup vs baseline: 1.2279x; 1.0670x over previous
"""MiniBatchDiscrimination kernel for 8 Trainium2 NeuronCores.

Reference computation (N=256 samples, A=2048 in_features, B=64 out_features,
C=32 kernel dim):
    M  = (f @ T).reshape(N, B, C)
    L1[i,j,b] = sum_c |M[j,b,c] - M[i,b,c]|
    o[j,b]    = sum_i exp(-L1[i,j,b])        (includes the i==j self term = 1)
    out = concat([f, o], axis=1)

Strategy (retrieval-knn pruning, see kernel_v1_backup.py for the full
derivation): ||v||_1 >= ||v||_2, so the squared-L2 screen
    D2[i,j,b] = n[i,b] + n[j,b] - 2*G[i,j,b]  (G = Gram of M_b)
with threshold T_SCREEN certifies every dropped pair contributes < 3e-15
to o.  For this input class the only survivors are the diagonal (count 1
== exact fp32 reference).  The host verifies (any o != 1 => exact
recompute of the affected columns), so the result is correct for ALL
inputs.

Sharding: tensor-parallel over the B*C columns of T: core d computes
o[:, 8d:8d+8] with no collectives.

v3 device pipeline per core (cost-model-guided):
  - f and T ship as fp8e4m3 partition-major.  Four loads ordered
    [fT(k0-7), Tb(half0), fT(k8-15), Tb(half1)] so half 0's GEMM (and its
    whole screen) starts one transfer earlier than half 1's.
  - GEMM M^T = (f @ Tblk)^T via DoubleRow fp8 matmuls (2 k-tiles per
    instruction, 0.5 cycles/row), one 128-row output half at a time.
  - per half t: ssb = M^2 (ACT square), msb = bf16 M (DVE copy); a
    single PSUM bank accumulates two norm-layout variants
      nppAB[:, 0:256]  rows 32g = n_b[j]/2 - T/2, rows 32g+1 = -1   (FJ)
      nppAB[:,256:512]  rows 32g = -1, rows 32g+1 = n_b[j]/2 - T/2  (FI)
    via S128a/S128b (0.5 selectors) + one const rank-1 each; FI/FJ are
    bf16 SBUF copies.
  - per (b): one [128, 2, 256] PSUM bank holds both i-half Grams, each
    fold-accumulated with the K=2 rank-1 [FI; FJ] so that
      Gff = G - n_i/2 - n_j/2 + T,   D2 < T  <=>  Gff > T/2 (constant!)
    One WIDE indicator op per b (DVE is_gt imm / ACT Sign imm bias) into
    an fp8 [128, 2, 256] tile; counts via one DoubleRow matmul per b
    (ones8 x ind8 -> [1, 256] at partition 32g of a per-half count bank).
  - count banks -> SBUF o8 [128, 512]; single 4-descriptor strided DMA.
"""

import os

import ml_dtypes
import numpy as np

N = 256  # batch
A = 2048  # in_features
B = 64  # out_features
C = 32  # kernel dim
NCORES = 8
BLOCAL = B // NCORES  # 8 b-features per core
BCL = BLOCAL * C  # 256 M^T rows per core
KT = A // 128  # 16 k-tiles
# Squared-L2 screen threshold: measured min off-diagonal computed D2 for
# fp8 f AND fp8 T is 1.64e4, 6.5x above T_SCREEN; identical rows compute
# D2 ~ 1e2 << T.  Computed D2 >= T still implies true L1 >= ~34.
T_SCREEN = 2500.0

_FP8 = ml_dtypes.float8_e4m3

# wide indicator engine per (t, g): 'D' = DVE is_gt, 'A' = ACT Sign.
# The op's accum column sums BOTH i-halves per partition: clean inputs
# give exactly 2.0 ('D') / -508.0 ('A') everywhere; any other value
# triggers the exact host fallback for that feature column.
_PATTERN = {0: ("D", "A", "D", "A"), 1: ("D", "A", "D", "D")}
ENG_ASSIGN = {(t, g): _PATTERN[t][g] for t in range(2) for g in range(4)}
CLEAN_VAL = {"D": 2.0, "A": 2.0 - 510.0}

_compiled = None
last_run_info = None


def _emit_body(nc, mybir, inp, work, scr, pbig, pn, consts, fT_d, Tb_d, o_d):
    f32 = mybir.dt.float32
    bf16 = mybir.dt.bfloat16
    fp8 = mybir.dt.float8e4
    S128a_sb, S128b_sb, cfa_sb, cfb_sb, ones_sb, biasA_sb = consts

    # ---- input loads: [fT0, TbH0, fT1, TbH1] all on the SP HWDGE queue so
    # the shared-HWDGE issue order matches the desired transfer order ----
    fT_ch, Tb_ch = [], []
    for c in range(2):
        ftt = inp.tile([128, 8, N], fp8, tag=f"fT{c}", name=f"ftt{c}")
        tbt = inp.tile([128, KT, 128], fp8, tag=f"Tb{c}", name=f"tbt{c}")
        fT_ch.append(ftt)
        Tb_ch.append(tbt)
    nc.sync.dma_start(fT_ch[0][:], fT_d[:, 0 : 8 * N])
    nc.sync.dma_start(Tb_ch[0][:], Tb_d[:, 0 : KT * 128])
    nc.sync.dma_start(fT_ch[1][:], fT_d[:, 8 * N : 16 * N])
    nc.sync.dma_start(Tb_ch[1][:], Tb_d[:, KT * 128 : 2 * KT * 128])

    # PE pstate warmup: start the ramp clock early
    wp = pn.tile([128, 128], f32, tag="pn", name="wp", bufs=1)
    for w in range(8):
        nc.tensor.matmul(
            wp[:],
            ones_sb[0:1, 0:128],
            ones_sb[0:1, 0:128],
            start=(w == 0),
            stop=(w == 7),
        )

    o_sb = work.tile([128, 2, 4], f32, tag="osb")

    def emit_gemm(t):
        mtp = pbig.tile([128, N], f32, tag="mtp", bufs=2, name=f"mtp{t}")
        for j in range(KT // 2):
            c = j // 4
            jj = j % 4
            nc.tensor.matmul(
                mtp[:],
                Tb_ch[t][:, 2 * j : 2 * j + 2, :],
                fT_ch[c][:, 2 * jj : 2 * jj + 2, :],
                start=(j == 0),
                stop=(j == KT // 2 - 1),
                perf_mode=mybir.MatmulPerfMode.DoubleRow,
            )
        return mtp

    def emit_norm_vec(t, mtp):
        # squares and bf16 M (both read the GEMM PSUM, one engine each)
        ssb = scr.tile([128, N], bf16, tag=f"sq{t}", name=f"ssb{t}")
        nc.scalar.square(ssb[:], mtp[:])
        msb = scr.tile([128, N], bf16, tag=f"mt{t}", name=f"msb{t}")
        nc.vector.tensor_copy(msb[:], mtp[:])
        return ssb, msb

    def emit_npp(t, ssb):
        # one PSUM bank, two layouts: FJ cols 0:256, FI cols 256:512
        nppAB = pn.tile([128, 2, N], f32, tag="pn", bufs=1, name=f"npp{t}")
        nc.tensor.matmul(
            nppAB[:, 0, :], S128a_sb[:], ssb[:], start=True, stop=False,
            skip_group_check=True,
        )
        nc.tensor.matmul(
            nppAB[:, 0, :], cfa_sb[0:1, :], ones_sb[0:1, 0:N],
            start=False, stop=False, skip_group_check=True,
        )
        nc.tensor.matmul(
            nppAB[:, 1, :], S128b_sb[:], ssb[:], start=False, stop=False,
            skip_group_check=True,
        )
        nc.tensor.matmul(
            nppAB[:, 1, :], cfb_sb[0:1, :], ones_sb[0:1, 0:N],
            start=False, stop=True, skip_group_check=True,
        )
        return nppAB

    def emit_fifj(t, nppAB):
        FJ = work.tile([128, N], bf16, tag=f"FJ{t}", name=f"FJ{t}")
        nc.scalar.activation(FJ[:], nppAB[:, 0, :], mybir.ActivationFunctionType.Copy)
        FI = work.tile([128, N], bf16, tag=f"FI{t}", name=f"FI{t}")
        nc.vector.tensor_copy(FI[:], nppAB[:, 1, :])
        return FI, FJ

    def emit_screen(t, msb, FI, FJ):
        for g in range(4):
            # both i-half Grams + K=2 folds share one PSUM bank; the first
            # matmul's start zeroes the whole bank (partition-scoped), so
            # the second half accumulates from zero with start=False.
            gp2 = pbig.tile([128, 2, N], f32, tag="big", bufs=3)
            for mt in range(2):
                nc.tensor.matmul(
                    gp2[:, mt, :],
                    msb[32 * g : 32 * g + 32, 128 * mt : 128 * (mt + 1)],
                    msb[32 * g : 32 * g + 32, :],
                    start=(mt == 0),
                    stop=False,
                    tile_position=(32 * g, 0),
                    skip_group_check=True,
                )
                nc.tensor.matmul(
                    gp2[:, mt, :],
                    FI[32 * g : 32 * g + 2, 128 * mt : 128 * (mt + 1)],
                    FJ[32 * g : 32 * g + 2, :],
                    start=False,
                    stop=(mt == 1),
                    tile_position=(32 * g, 0),
                    skip_group_check=True,
                )
            ind8 = scr.tile([128, 2, N], fp8, tag="ind", name="ind")
            if ENG_ASSIGN[(t, g)] == "A":
                # sign(Gff - T/2) = +1 iff D2 < T; clean accum = -508
                nc.scalar.activation(
                    ind8[:],
                    gp2[:],
                    mybir.ActivationFunctionType.Sign,
                    bias=biasA_sb[:, 0:1],
                    scale=1.0,
                    accum_out=o_sb[:, t, g : g + 1],
                )
            else:
                nc.vector.tensor_scalar(
                    ind8[:],
                    gp2[:],
                    T_SCREEN / 2.0,
                    None,
                    mybir.AluOpType.is_gt,
                    mybir.AluOpType.add,
                    accum_out=o_sb[:, t, g : g + 1],
                )

    mtps = [emit_gemm(0), emit_gemm(1)]
    sm0 = emit_norm_vec(0, mtps[0])
    sm1 = emit_norm_vec(1, mtps[1])
    npp0 = emit_npp(0, sm0[0])
    fifj0 = emit_fifj(0, npp0)
    npp1 = emit_npp(1, sm1[0])
    fifj1 = emit_fifj(1, npp1)
    emit_screen(0, sm0[1], fifj0[0], fifj0[1])
    emit_screen(1, sm1[1], fifj1[0], fifj1[1])

    nc.sync.dma_start(o_d[:], o_sb[:])


def _build():
    import concourse.mybir as mybir
    import concourse.tile as tile
    from concourse import bacc

    f32 = mybir.dt.float32
    bf16 = mybir.dt.bfloat16
    fp8 = mybir.dt.float8e4

    nc = bacc.Bacc(None, target_bir_lowering=False, debug=False)
    fT_d = nc.dram_tensor("fT", [128, KT * N], fp8, kind="ExternalInput")
    Tb_d = nc.dram_tensor("Tb", [128, 2 * KT * 128], fp8, kind="ExternalInput")
    o_d = nc.dram_tensor("o", [128, 8], f32, kind="ExternalOutput")

    with tile.TileContext(nc) as tc:
        with (
            tc.tile_pool(name="inp", bufs=1) as inp,
            tc.tile_pool(name="work", bufs=1) as work,
            tc.tile_pool(name="scr", bufs=2) as scr,
            tc.tile_pool(name="pbig", bufs=1, space="PSUM") as pbig,
            tc.tile_pool(name="pn", bufs=1, space="PSUM") as pn,
        ):
            # S128a: 0.5 at (rows of g, col 32g); S128b: col 32g+1
            S128a_sb = work.tile([128, 128], bf16, tag="S128a")
            nc.vector.memset(S128a_sb[:], 0.0)
            S128b_sb = work.tile([128, 128], bf16, tag="S128b")
            nc.vector.memset(S128b_sb[:], 0.0)
            for g in range(4):
                nc.vector.memset(
                    S128a_sb[32 * g : 32 * g + 32, 32 * g : 32 * g + 1], 0.5
                )
                nc.vector.memset(
                    S128b_sb[32 * g : 32 * g + 32, 32 * g + 1 : 32 * g + 2], 0.5
                )
            # const rank-1 rows: cfa cols 32g = -T/2, cols 32g+1 = -1;
            # cfb swapped
            cfa_sb = work.tile([1, 128], bf16, tag="cfa")
            nc.gpsimd.memset(cfa_sb[:], 0.0)
            cfb_sb = work.tile([1, 128], bf16, tag="cfb")
            nc.gpsimd.memset(cfb_sb[:], 0.0)
            for g in range(4):
                nc.gpsimd.memset(
                    cfa_sb[0:1, 32 * g : 32 * g + 1], -T_SCREEN / 2.0
                )
                nc.gpsimd.memset(cfa_sb[0:1, 32 * g + 1 : 32 * g + 2], -1.0)
                nc.gpsimd.memset(cfb_sb[0:1, 32 * g : 32 * g + 1], -1.0)
                nc.gpsimd.memset(
                    cfb_sb[0:1, 32 * g + 1 : 32 * g + 2], -T_SCREEN / 2.0
                )
            ones_sb = work.tile([128, N], bf16, tag="ones")
            nc.vector.memset(ones_sb[:], 1.0)
            biasA_sb = work.tile([128, 1], f32, tag="biasA")
            nc.gpsimd.memset(biasA_sb[:], -T_SCREEN / 2.0)

            _emit_body(
                nc, mybir, inp, work, scr, pbig, pn,
                (S128a_sb, S128b_sb, cfa_sb, cfb_sb, ones_sb, biasA_sb),
                fT_d, Tb_d, o_d,
            )

    nc.compile()
    return nc


def _get_compiled():
    global _compiled
    if _compiled is None:
        _compiled = _build()
    return _compiled


def _host_exact_o_column(f64, T64, b):
    """Exact (float64) o[:, b] for one feature column; used only when the
    device screen detects a potential near-duplicate pair."""
    Mb = f64 @ T64[:, C * b : C * (b + 1)]  # (N, C)
    L1 = np.abs(Mb[None, :, :] - Mb[:, None, :]).sum(axis=2)  # (N, N)
    return np.exp(-L1).sum(axis=0)


def _tile_rows(x):
    """(A, W) row-major -> (128, KT*W) partition-major (row p = k-tiles concat)."""
    w = x.shape[1]
    return np.ascontiguousarray(
        x.reshape(KT, 128, w).transpose(1, 0, 2).reshape(128, KT * w)
    )


def make_in_maps(f, T):
    fT = _tile_rows(f.T.astype(_FP8))
    maps = []
    for d in range(NCORES):
        Tb = T[:, BCL * d : BCL * (d + 1)].astype(_FP8)  # (2048, 256)
        # half-major: [128p, half, kt, 128cols]
        Tb4 = Tb.reshape(KT, 128, 2, 128).transpose(1, 2, 0, 3)
        maps.append(
            {"fT": fT, "Tb": np.ascontiguousarray(Tb4).reshape(128, 2 * KT * 128)}
        )
    return maps


def kernel(f, T):
    from concourse.bass_utils import run_bass_kernel_spmd

    global last_run_info
    f = np.asarray(f)
    T = np.asarray(T)
    assert f.shape == (N, A) and T.shape == (A, B * C), (f.shape, T.shape)

    nc = _get_compiled()
    in_maps = make_in_maps(f, T)
    res = run_bass_kernel_spmd(
        nc,
        in_maps,
        core_ids=list(range(NCORES)),
        trace=bool(int(os.environ.get("KERNEL_TRACE", "0"))),
    )
    last_run_info = res

    # Device returns, per (t, g), the per-partition accum over BOTH i-halves
    # and all j: clean inputs give exactly CLEAN_VAL everywhere.  Any other
    # value (near-duplicate pair somewhere in that feature column) => exact
    # host recompute of the column.
    o = np.ones((N, B), dtype=np.float32)
    bad = []
    for d in range(NCORES):
        od = np.array(res.results[d]["o"]).reshape(128, 2, 4)  # [p, t, g]
        for t in range(2):
            for g in range(4):
                if np.any(od[:, t, g] != CLEAN_VAL[ENG_ASSIGN[(t, g)]]):
                    bad.append(BLOCAL * d + 4 * t + g)
    if bad:
        f64 = f.astype(np.float64)
        T64 = T.astype(np.float64)
        for b in bad:
            o[:, b] = _host_exact_o_column(f64, T64, int(b)).astype(np.float32)

    return np.concatenate([f.astype(np.float32, copy=False), o], axis=1)


# revision 34
# speedup vs baseline: 1.3499x; 1.0993x over previous
"""MiniBatchDiscrimination kernel for 8 Trainium2 NeuronCores.

Reference computation (N=256 samples, A=2048 in_features, B=64 out_features,
C=32 kernel dim):
    M  = (f @ T).reshape(N, B, C)
    L1[i,j,b] = sum_c |M[j,b,c] - M[i,b,c]|
    o[j,b]    = sum_i exp(-L1[i,j,b])        (includes the i==j self term = 1)
    out = concat([f, o], axis=1)

Strategy (retrieval-knn pruning, see kernel_v1_backup.py for the full
derivation): ||v||_1 >= ||v||_2, so the squared-L2 screen
    D2[i,j,b] = n[i,b] + n[j,b] - 2*G[i,j,b]  (G = Gram of M_b)
with threshold T_SCREEN certifies every dropped pair contributes < 3e-15
to o.  For this input class the only survivors are the diagonal (count 1
== exact fp32 reference).  The host verifies (any o != 1 => exact
recompute of the affected columns), so the result is correct for ALL
inputs.

Sharding: tensor-parallel over the B*C columns of T: core d computes
o[:, 8d:8d+8] with no collectives.

v3 device pipeline per core (cost-model-guided):
  - f and T ship as fp8e4m3 partition-major.  Four loads ordered
    [fT(k0-7), Tb(half0), fT(k8-15), Tb(half1)] so half 0's GEMM (and its
    whole screen) starts one transfer earlier than half 1's.
  - GEMM M^T = (f @ Tblk)^T via DoubleRow fp8 matmuls (2 k-tiles per
    instruction, 0.5 cycles/row), one 128-row output half at a time.
  - per half t: ssb = M^2 (ACT square), msb = bf16 M (DVE copy); a
    single PSUM bank accumulates two norm-layout variants
      nppAB[:, 0:256]  rows 32g = n_b[j]/2 - T/2, rows 32g+1 = -1   (FJ)
      nppAB[:,256:512]  rows 32g = -1, rows 32g+1 = n_b[j]/2 - T/2  (FI)
    via S128a/S128b (0.5 selectors) + one const rank-1 each; FI/FJ are
    bf16 SBUF copies.
  - per (b): one [128, 2, 256] PSUM bank holds both i-half Grams, each
    fold-accumulated with the K=2 rank-1 [FI; FJ] so that
      Gff = G - n_i/2 - n_j/2 + T,   D2 < T  <=>  Gff > T/2 (constant!)
    One WIDE indicator op per b (DVE is_gt imm / ACT Sign imm bias) into
    an fp8 [128, 2, 256] tile; counts via one DoubleRow matmul per b
    (ones8 x ind8 -> [1, 256] at partition 32g of a per-half count bank).
  - count banks -> SBUF o8 [128, 512]; single 4-descriptor strided DMA.
"""

import os

import ml_dtypes
import numpy as np

N = 256  # batch
A = 2048  # in_features
B = 64  # out_features
C = 32  # kernel dim
NCORES = 8
BLOCAL = B // NCORES  # 8 b-features per core
BCL = BLOCAL * C  # 256 M^T rows per core
KT = A // 128  # 16 k-tiles
# Squared-L2 screen threshold: measured min off-diagonal computed D2 for
# fp8 f AND fp8 T is 1.64e4, 6.5x above T_SCREEN; identical rows compute
# D2 ~ 1e2 << T.  Computed D2 >= T still implies true L1 >= ~34.
T_SCREEN = 2500.0

_FP8 = ml_dtypes.float8_e4m3

# wide indicator engine per (t, g): 'D' = DVE is_gt, 'A' = ACT Sign.
# The op's accum column sums BOTH i-halves per partition: clean inputs
# give exactly 2.0 ('D') / -508.0 ('A') everywhere; any other value
# triggers the exact host fallback for that feature column.
_PATTERN = {0: ("D", "A", "D", "A"), 1: ("A", "D", "A", "D")}
ENG_ASSIGN = {(t, g): _PATTERN[t][g] for t in range(2) for g in range(4)}
CLEAN_VAL = {"D": 2.0, "A": 2.0 - 510.0}

_compiled = None
last_run_info = None


def _emit_body(nc, mybir, inp, work, scr, pbig, pn, consts, fT_d, Tb_d, o_d):
    f32 = mybir.dt.float32
    bf16 = mybir.dt.bfloat16
    fp8 = mybir.dt.float8e4
    S128a_sb, S128b_sb, cfa_sb, cfb_sb, ones_sb, biasA_sb = consts

    # ---- input loads: [fT0, TbH0, fT1, TbH1] all on the SP HWDGE queue so
    # the shared-HWDGE issue order matches the desired transfer order ----
    fT_ch, Tb_ch = [], []
    for c in range(2):
        ftt = inp.tile([128, 8, N], fp8, tag=f"fT{c}", name=f"ftt{c}")
        tbt = inp.tile([128, KT, 128], fp8, tag=f"Tb{c}", name=f"tbt{c}")
        fT_ch.append(ftt)
        Tb_ch.append(tbt)
    nc.sync.dma_start(fT_ch[0][:], fT_d[:, 0 : 8 * N])
    nc.sync.dma_start(Tb_ch[0][:], Tb_d[:, 0 : KT * 128])
    nc.sync.dma_start(fT_ch[1][:], fT_d[:, 8 * N : 16 * N])
    nc.sync.dma_start(Tb_ch[1][:], Tb_d[:, KT * 128 : 2 * KT * 128])
    # pre-zero the (padded) output region; lands well before the scatter fires
    zz = work.tile([128, 64], f32, tag="zz")
    nc.vector.memset(zz[:], 0.0)
    nc.sync.dma_start(o_d[:], zz[:])

    # PE pstate warmup: start the ramp clock early
    wp = pn.tile([128, 128], f32, tag="pn", name="wp", bufs=1)
    for w in range(8):
        nc.tensor.matmul(
            wp[:],
            ones_sb[0:1, 0:128],
            ones_sb[0:1, 0:128],
            start=(w == 0),
            stop=(w == 7),
        )

    # accum columns live in the first 8 of a padded 64-f32 scatter payload
    o_sb = work.tile([128, 1, 64], f32, tag="osb")
    nc.vector.memset(o_sb[:], 0.0)
    idxs = work.tile([16, 8], mybir.dt.int16, tag="idxs")
    nc.gpsimd.iota(idxs[:], [[16, 8]], base=0, channel_multiplier=1)
    dma_sem = nc.alloc_semaphore(name="oscat")
    nc.gpsimd.dma_scatter_add(
        o_d[:],
        o_sb[:],
        idxs[:],
        num_idxs=128,
        num_idxs_reg=128,
        elem_size=64,
        prepare_only=True,
        sem=dma_sem,
    )

    def emit_gemm(t):
        mtp = pbig.tile([128, N], f32, tag="mtp", bufs=2, name=f"mtp{t}")
        for j in range(KT // 2):
            c = j // 4
            jj = j % 4
            nc.tensor.matmul(
                mtp[:],
                Tb_ch[t][:, 2 * j : 2 * j + 2, :],
                fT_ch[c][:, 2 * jj : 2 * jj + 2, :],
                start=(j == 0),
                stop=(j == KT // 2 - 1),
                perf_mode=mybir.MatmulPerfMode.DoubleRow,
            )
        return mtp

    def emit_norm_vec(t, mtp):
        # single PSUM read (two engines reading one PSUM tile serialize);
        # squares derive from the bf16 copy at DVE 4x rate
        msb = scr.tile([128, N], bf16, tag=f"mt{t}", name=f"msb{t}")
        nc.vector.tensor_copy(msb[:], mtp[:])
        ssb = scr.tile([128, N], bf16, tag=f"sq{t}", name=f"ssb{t}")
        nc.vector.tensor_tensor(ssb[:], msb[:], msb[:], mybir.AluOpType.mult)
        return ssb, msb

    def emit_npp(t, ssb):
        # one PSUM bank, two layouts: FJ cols 0:256, FI cols 256:512
        nppAB = pn.tile([128, 2, N], f32, tag="pn", bufs=1, name=f"npp{t}")
        nc.tensor.matmul(
            nppAB[:, 0, :], S128a_sb[:], ssb[:], start=True, stop=False,
            skip_group_check=True,
        )
        nc.tensor.matmul(
            nppAB[:, 0, :], cfa_sb[0:1, :], ones_sb[0:1, 0:N],
            start=False, stop=False, skip_group_check=True,
        )
        nc.tensor.matmul(
            nppAB[:, 1, :], S128b_sb[:], ssb[:], start=False, stop=False,
            skip_group_check=True,
        )
        nc.tensor.matmul(
            nppAB[:, 1, :], cfb_sb[0:1, :], ones_sb[0:1, 0:N],
            start=False, stop=True, skip_group_check=True,
        )
        return nppAB

    def emit_fifj(t, nppAB):
        FJ = work.tile([128, N], bf16, tag=f"FJ{t}", name=f"FJ{t}")
        nc.scalar.activation(FJ[:], nppAB[:, 0, :], mybir.ActivationFunctionType.Copy)
        FI = work.tile([128, N], bf16, tag=f"FI{t}", name=f"FI{t}")
        if t == 0:
            nc.scalar.activation(
                FI[:], nppAB[:, 1, :], mybir.ActivationFunctionType.Copy
            )
        else:
            nc.vector.tensor_copy(FI[:], nppAB[:, 1, :])
        return FI, FJ

    def emit_screen(t, msb, FI, FJ):
        # grams first (only need msb), folds + indicators after (need FI/FJ)
        gp2s = []
        for g in range(4):
            # both i-half Grams + K=2 folds share one PSUM bank; the first
            # matmul's start zeroes the whole bank (partition-scoped), so
            # the second half accumulates from zero with start=False.
            gp2 = pbig.tile([128, 2, N], f32, tag="big", bufs=4)
            gp2s.append(gp2)
            for mt in range(2):
                nc.tensor.matmul(
                    gp2[:, mt, :],
                    msb[32 * g : 32 * g + 32, 128 * mt : 128 * (mt + 1)],
                    msb[32 * g : 32 * g + 32, :],
                    start=(mt == 0),
                    stop=False,
                    tile_position=(32 * g, 0),
                    skip_group_check=True,
                )
        for g in range(4):
            gp2 = gp2s[g]
            for mt in range(2):
                nc.tensor.matmul(
                    gp2[:, mt, :],
                    FI[32 * g : 32 * g + 2, 128 * mt : 128 * (mt + 1)],
                    FJ[32 * g : 32 * g + 2, :],
                    start=False,
                    stop=(mt == 1),
                    tile_position=(32 * g, 0),
                    skip_group_check=True,
                )
            col = 4 * t + g
            ind8 = scr.tile([128, 2, N], fp8, tag="ind", name="ind")
            if ENG_ASSIGN[(t, g)] == "A":
                # sign(Gff - T/2) = +1 iff D2 < T; clean accum = -508
                nc.scalar.activation(
                    ind8[:],
                    gp2[:],
                    mybir.ActivationFunctionType.Sign,
                    bias=biasA_sb[:, 0:1],
                    scale=1.0,
                    accum_out=o_sb[:, 0, col : col + 1],
                )
            else:
                nc.vector.tensor_scalar(
                    ind8[:],
                    gp2[:],
                    T_SCREEN / 2.0,
                    None,
                    mybir.AluOpType.is_gt,
                    mybir.AluOpType.add,
                    accum_out=o_sb[:, 0, col : col + 1],
                )

    mtps = [emit_gemm(0), emit_gemm(1)]
    sm0 = emit_norm_vec(0, mtps[0])
    sm1 = emit_norm_vec(1, mtps[1])
    npp0 = emit_npp(0, sm0[0])
    fifj0 = emit_fifj(0, npp0)
    npp1 = emit_npp(1, sm1[0])
    fifj1 = emit_fifj(1, npp1)
    emit_screen(0, sm0[1], fifj0[0], fifj0[1])
    emit_screen(1, sm1[1], fifj1[0], fifj1[1])

    # fire the prepared scatter; Tile moves the o_sb data deps here
    nc.gpsimd.trigger_dma(count=None)


def _build():
    import concourse.mybir as mybir
    import concourse.tile as tile
    from concourse import bacc

    f32 = mybir.dt.float32
    bf16 = mybir.dt.bfloat16
    fp8 = mybir.dt.float8e4

    nc = bacc.Bacc(None, target_bir_lowering=False, debug=False)
    fT_d = nc.dram_tensor("fT", [128, KT * N], fp8, kind="ExternalInput")
    Tb_d = nc.dram_tensor("Tb", [128, 2 * KT * 128], fp8, kind="ExternalInput")
    o_d = nc.dram_tensor("o", [128, 64], f32, kind="ExternalOutput")

    with tile.TileContext(nc) as tc:
        with (
            tc.tile_pool(name="inp", bufs=1) as inp,
            tc.tile_pool(name="work", bufs=1) as work,
            tc.tile_pool(name="scr", bufs=2) as scr,
            tc.tile_pool(name="pbig", bufs=1, space="PSUM") as pbig,
            tc.tile_pool(name="pn", bufs=1, space="PSUM") as pn,
        ):
            # S128a: 0.5 at (rows of g, col 32g); S128b: col 32g+1
            S128a_sb = work.tile([128, 128], bf16, tag="S128a")
            nc.vector.memset(S128a_sb[:], 0.0)
            S128b_sb = work.tile([128, 128], bf16, tag="S128b")
            nc.vector.memset(S128b_sb[:], 0.0)
            for g in range(4):
                nc.vector.memset(
                    S128a_sb[32 * g : 32 * g + 32, 32 * g : 32 * g + 1], 0.5
                )
                nc.vector.memset(
                    S128b_sb[32 * g : 32 * g + 32, 32 * g + 1 : 32 * g + 2], 0.5
                )
            # const rank-1 rows: cfa cols 32g = -T/2, cols 32g+1 = -1;
            # cfb swapped
            cfa_sb = work.tile([1, 128], bf16, tag="cfa")
            nc.gpsimd.memset(cfa_sb[:], 0.0)
            cfb_sb = work.tile([1, 128], bf16, tag="cfb")
            nc.gpsimd.memset(cfb_sb[:], 0.0)
            for g in range(4):
                nc.gpsimd.memset(
                    cfa_sb[0:1, 32 * g : 32 * g + 1], -T_SCREEN / 2.0
                )
                nc.gpsimd.memset(cfa_sb[0:1, 32 * g + 1 : 32 * g + 2], -1.0)
                nc.gpsimd.memset(cfb_sb[0:1, 32 * g : 32 * g + 1], -1.0)
                nc.gpsimd.memset(
                    cfb_sb[0:1, 32 * g + 1 : 32 * g + 2], -T_SCREEN / 2.0
                )
            ones_sb = work.tile([128, N], bf16, tag="ones")
            nc.vector.memset(ones_sb[:], 1.0)
            biasA_sb = work.tile([128, 1], f32, tag="biasA")
            nc.gpsimd.memset(biasA_sb[:], -T_SCREEN / 2.0)

            _emit_body(
                nc, mybir, inp, work, scr, pbig, pn,
                (S128a_sb, S128b_sb, cfa_sb, cfb_sb, ones_sb, biasA_sb),
                fT_d, Tb_d, o_d,
            )

    nc.compile()

    # Tile's end-of-program drain accounts the prepared scatter on the DMASW0
    # lane, but a gen_mode==1 prep signals its completion through the explicit
    # `sem=` (oscat) instead — the DMASW0 wait would deadlock.  Remap those
    # waits to the real completion sem (same +16, same semantics).
    oscat = None
    for inst in nc.inst_map.values():
        si = inst.sync_info
        if si is None:
            continue
        for u in si.on_update:
            if u.ant_name == "oscat":
                oscat = (u.id, u.ant_name)
    assert oscat is not None
    for inst in nc.inst_map.values():
        si = inst.sync_info
        if si is None or not si.on_wait:
            continue
        if any(w.ant_name and w.ant_name.startswith("DMASW") for w in si.on_wait):
            new_waits = [
                mybir.SyncWait(
                    sync_type="semaphore",
                    id=oscat[0],
                    ant_name=oscat[1],
                    wait_mode="sem-ge-imm",
                    wait_value=16,
                    wait_reg=None,
                )
                if (w.ant_name and w.ant_name.startswith("DMASW"))
                else w
                for w in si.on_wait
            ]
            inst.sync_info = mybir.SyncInfo(
                on_wait=new_waits, on_update=list(si.on_update)
            )
    return nc


def _get_compiled():
    global _compiled
    if _compiled is None:
        _compiled = _build()
    return _compiled


def _host_exact_o_column(f64, T64, b):
    """Exact (float64) o[:, b] for one feature column; used only when the
    device screen detects a potential near-duplicate pair."""
    Mb = f64 @ T64[:, C * b : C * (b + 1)]  # (N, C)
    L1 = np.abs(Mb[None, :, :] - Mb[:, None, :]).sum(axis=2)  # (N, N)
    return np.exp(-L1).sum(axis=0)


def _tile_rows(x):
    """(A, W) row-major -> (128, KT*W) partition-major (row p = k-tiles concat)."""
    w = x.shape[1]
    return np.ascontiguousarray(
        x.reshape(KT, 128, w).transpose(1, 0, 2).reshape(128, KT * w)
    )


def make_in_maps(f, T):
    fT = _tile_rows(f.T.astype(_FP8))
    maps = []
    for d in range(NCORES):
        Tb = T[:, BCL * d : BCL * (d + 1)].astype(_FP8)  # (2048, 256)
        # half-major: [128p, half, kt, 128cols]
        Tb4 = Tb.reshape(KT, 128, 2, 128).transpose(1, 2, 0, 3)
        maps.append(
            {"fT": fT, "Tb": np.ascontiguousarray(Tb4).reshape(128, 2 * KT * 128)}
        )
    return maps


def kernel(f, T):
    from concourse.bass_utils import run_bass_kernel_spmd

    global last_run_info
    f = np.asarray(f)
    T = np.asarray(T)
    assert f.shape == (N, A) and T.shape == (A, B * C), (f.shape, T.shape)

    nc = _get_compiled()
    in_maps = make_in_maps(f, T)
    res = run_bass_kernel_spmd(
        nc,
        in_maps,
        core_ids=list(range(NCORES)),
        trace=bool(int(os.environ.get("KERNEL_TRACE", "0"))),
    )
    last_run_info = res

    # Device returns, per (t, g), the per-partition accum over BOTH i-halves
    # and all j: clean inputs give exactly CLEAN_VAL everywhere.  Any other
    # value (near-duplicate pair somewhere in that feature column) => exact
    # host recompute of the column.
    o = np.ones((N, B), dtype=np.float32)
    bad = []
    for d in range(NCORES):
        od = np.array(res.results[d]["o"])[:, :8].reshape(128, 2, 4)  # [p, t, g]
        for t in range(2):
            for g in range(4):
                if np.any(od[:, t, g] != CLEAN_VAL[ENG_ASSIGN[(t, g)]]):
                    bad.append(BLOCAL * d + 4 * t + g)
    if bad:
        f64 = f.astype(np.float64)
        T64 = T.astype(np.float64)
        for b in bad:
            o[:, b] = _host_exact_o_column(f64, T64, int(b)).astype(np.float32)

    return np.concatenate([f.astype(np.float32, copy=False), o], axis=1)
